# revision 63
# baseline (speedup 1.0000x reference)
"""Angular (arccos-power) attention on 8 Trainium2 NeuronCores.

Sharding: core c in 0..7 -> batch b = c//4, head-group g = c%4 (4 of 16 heads).
Each core computes its 4 heads' attention for its batch over the full
sequence, plus the partial out-projection for its head slice. The host
sums the 4 per-group partials per batch and adds the output bias.

Device math (HW-validated vs the jax reference at 2.05e-3 global rel err):
  raw[s,t]   = k_s . (q_t * rq_t)          (rq applied via the per-partition
                                            scale slot of the Q evacuation
                                            copy, natural layout: partition==t)
  a          = arctan(rk_s * raw)          (rk folded into the Atan scale slot,
                                            output in fp16)
  g(a)       = arcsin(tan a) = a*(1 + u*h(u)),  u = a^2, h deg-3 poly
  base       = pi/2 + g(a)   ( = arccos(-cos_sim) )
  w          = base^16       (4 fused squarings; the pi^-16 and row
                              normalization cancel in w / sum_s w)
  out        = (w^T @ [V|1]) -> per-row divide -> @ Wo_slice^T (bf16)

Per score element: one ScalarE pass (Arctan) + two fused custom-DVE passes
(ANG_GPOLY, ANG_POW16) -- no activation-table switches. Custom DVE ops run
REGULAR mode (1 elem/cyc/partition; the 2x modes need a doubled uop
program that cannot fit these 6-8-op bodies in the 8-stage pipeline), so
the two DVE passes over the T^2 scores are the bottleneck engine.
Everything feasible is moved off the DVE/ScalarE critical pair:
  - per-head sum-of-squares: one wide Act Square + paged DVE X-reduce
  - Q/K transposes go through the psO psum pool so psA triple-buffers
    the projection matmuls; their evacuations run on the DVE
  - A@V row-normalize runs on the idle GPSIMD engine (normalize_recip)
  - o-chain (o_norm/oT/Wo) in bf16; o-transposes use the 1 cyc/row
    bf16 transpose path instead of a 4 cyc/row narrow f32r matmul;
    the out-projection evacuation runs on the DVE
An optional Act-assisted path (ND>0: z-poly on DVE + Ln/Exp on ScalarE,
act tables batched per head) measured slower on HW; ND=0 disables it.

Cost-model exec: 439 us/8-core pass; paired differential HW measurement
(compile-once, strictly alternating reps=1/reps=10 executions, 150
pairs) gives ~237 us median (90% CI ~[219, 274]); the same rig measures
the previous session's baseline at 369 us (its claimed 202 us was
noise). Evidence across variants (Act-load cuts move HW time ~4x more
than the model predicts; shifting DVE work onto Act hurts HW) indicates
the ScalarE/Activation engine is the real silicon bottleneck, with the
custom-DVE passes running faster than the model's 1 elem/cyc pricing.
`build_nc(T, reps)` emits the pipeline `reps` times for measurement;
the grading path uses reps=1.
"""

import numpy as np

# h(u) coefficients for g(a) = a*(1 + u*h(u)), u = a^2, |a| <= atan(0.72)
_HC = [0.49966321378304285, 0.4003792826334919,
       0.06195379198923147, 1.2597252842980673]
# deg-3 seed for 1/sqrt(ss), ss in [9, 62] (two Newton steps follow)
_RC = [0.4423299131475817, -0.01588131025257223,
       0.00029869448025181695, -2.0168811221534655e-06]
_PI_2 = float(np.pi / 2)
# deg-2 h for the D-path z-op: z = a*(1 + u*h(u)) ~= arcsin(tan a),
# |a| <= atan(0.70) (data max |cos| = 0.643); max abs z err 1.2e-4
_ZC = [0.5183908311554029, 0.1555597370285856, 1.0492311247265849]

_OPS = None
_BUILT = {}


def _ensure_ops():
    """Register the four fused DVE ops (idempotent)."""
    global _OPS
    if _OPS is not None:
        return _OPS
    from concourse import dve_ops
    from concourse.dve_spec import (
        Spec, Src0, Src1, C0, C1, C2, C3, One, lower, sq,
        _spill_c3_to_src1, _has_src1,
    )
    from concourse.dve_uop import DveOpSpec

    existing = {op.name: op for op in dve_ops.OPS}
    if "ANG_SQ" in existing:
        _OPS = existing
        return _OPS

    f32 = np.float32

    def _ref_gpoly(in0, in1, s0, s1, imm2):
        u = (in0.astype(f32) * in0).astype(f32)
        h = (((in1 * u + imm2) * u + s1) * u + s0).astype(f32)
        return (u * h).astype(f32)

    def _ref_pow16(in0, in1, s0, s1, imm2):
        b = (s0 + in0.astype(f32) * (1.0 + in1)).astype(f32)
        b = (b * b).astype(f32)
        b = (b * b).astype(f32)
        b = (b * b).astype(f32)
        return (b * b).astype(f32)

    def _ref_rsqseed(in0, in1, s0, s1, imm2):
        x = in0.astype(f32)
        return (((in1 * x + imm2) * x + s1) * x + s0).astype(f32)

    def _ref_rsqnr(in0, in1, s0, s1, imm2):
        y = in0.astype(f32)
        return (y * (s0 - s1 * in1 * y * y)).astype(f32)

    def _ref_sq(in0, in1, s0, s1, imm2):
        x = in0.astype(f32)
        return (x * x).astype(f32)

    def _ref_zpoly(in0, in1, s0, s1, imm2):
        a = in0.astype(f32)
        u = (a * a).astype(f32)
        t = ((imm2 * u + s1) * u + s0).astype(f32)
        return (a * (1.0 + u * t)).astype(f32)

    u = sq(Src0)
    h = ((C3 * u + C2) * u + C1) * u + C0
    zsrc = Src0
    zu = sq(zsrc)
    defs = [
        ("ANG_GPOLY", _spill_c3_to_src1(u * h), _ref_gpoly),
        ("ANG_POW16", sq(sq(sq(sq(C0 + Src0 * (One + Src1))))), _ref_pow16),
        ("ANG_RSQSEED",
         _spill_c3_to_src1(((C3 * Src0 + C2) * Src0 + C1) * Src0 + C0),
         _ref_rsqseed),
        ("ANG_RSQNR", Src0 * (C0 - C1 * Src1 * sq(Src0)), _ref_rsqnr),
        ("ANG_SQ", sq(Src0), _ref_sq),
        ("ANG_ZPOLY", zsrc * (One + ((C2 * zu + C1) * zu + C0) * zu), _ref_zpoly),
    ]
    for name, body, ref in defs:
        spec = Spec(body=body, reference=ref)
        row = dve_ops._CUSTOM_DVE_ROW_BASE + len(dve_ops.OPS)
        shas = {}
        for ver in ("v3", "v4"):
            s = DveOpSpec(name=name, opcode=row,
                          uops=lower(spec, ver=ver), rd1_en=_has_src1(spec))
            shas[ver] = s.sha(ver)
        op = dve_ops.DveOp(name, spec, subdim=False, uops_sha=shas)
        dve_ops.OPS.append(op)
        dve_ops.CUSTOM_DVE_SPECS[name] = spec
        dve_ops._SUB_OPCODE_FOR_NAME[name] = row
    _OPS = {op.name: op for op in dve_ops.OPS}
    return _OPS


def build_nc(T=2048, reps=1):
    """Build the per-core Bass graph (identical on all 8 cores)."""
    from contextlib import ExitStack
    from concourse import bacc, bass, tile, mybir

    ops = _ensure_ops()
    GPOLY, POW16 = ops["ANG_GPOLY"], ops["ANG_POW16"]
    RSQSEED, RSQNR = ops["ANG_RSQSEED"], ops["ANG_RSQNR"]
    SQ = ops["ANG_SQ"]
    ZPOLY = ops["ANG_ZPOLY"]

    f32 = mybir.dt.float32
    f32r = mybir.dt.float32r
    bf16 = mybir.dt.bfloat16
    AF = mybir.ActivationFunctionType
    ts = bass.ts

    NT = T // 128          # t-chunks == s-strips
    NK = 8                 # d_in contraction tiles
    TW = 1024 if T >= 2048 else T
    NHALF = T // TW        # elementwise sub-strips per strip

    nc = bacc.Bacc(None, target_bir_lowering=False)

    xd = nc.declare_dram_parameter("xb", [NT, 128, NK * 128], f32r, isOutput=False)
    wqkd = nc.declare_dram_parameter("wqk", [128, NK * 512], f32r, isOutput=False)
    wvd = nc.declare_dram_parameter("wv", [128, NK * 256], f32r, isOutput=False)
    wod = nc.declare_dram_parameter("wo", [128, 2 * 1024], bf16, isOutput=False)
    eyed = nc.declare_dram_parameter("eye", [128, 128], f32r, isOutput=False)
    outd = nc.declare_dram_parameter("out", [T, 1024], f32, isOutput=True)

    with tile.TileContext(nc) as tc, ExitStack() as ctx:
        ep = ctx.enter_context  # noqa
        cw = ep(tc.tile_pool(name="const", bufs=1))
        recpool = ep(tc.tile_pool(name="rec", bufs=4))
        psA = ep(tc.tile_pool(name="psA", bufs=3, space=bass.MemorySpace.PSUM))
        psO = ep(tc.tile_pool(name="psO", bufs=2, space=bass.MemorySpace.PSUM))

        # ---- constants (shared across reps) ----
        wo_t = cw.tile([128, 2, 1024], bf16, tag="wo", name="wo")
        eye_t = cw.tile([128, 128], f32r, tag="eye", name="eye")
        eye_b = cw.tile([128, 128], bf16, tag="eyeb", name="eyeb")
        k4t = cw.tile([128, 1], f32, tag="k4", name="k4")
        rc3t = cw.tile([128, 1], f32, tag="rc3", name="rc3")
        pi2t = cw.tile([128, 1], f32, tag="pi2", name="pi2")
        nc.sync.dma_start(out=eye_t[:, :], in_=eyed[:, :])
        nc.vector.tensor_copy(eye_b[:, :], eye_t[:, :])
        nc.vector.memset(k4t[:], float(_HC[3]))
        nc.vector.memset(rc3t[:], float(_RC[3]))
        nc.vector.memset(pi2t[:], _PI_2)

        for _rep in range(reps):
            _emit_rep(nc, tc, ctx, _rep, T, NT, NK, TW, NHALF,
                      wqkd, wvd, wod, wo_t, eye_t, eye_b, k4t, rc3t, pi2t,
                      xd, outd, recpool, psA, psO,
                      GPOLY, POW16, RSQSEED, RSQNR, SQ, ZPOLY, AF, ts, f32, f32r, bf16)

    nc.compile()
    return nc


def _emit_rep(nc, tc, ctx, _rep, T, NT, NK, TW, NHALF,
              wqkd, wvd, wod, wo_t, eye_t, eye_b, k4t, rc3t, pi2t,
              xd, outd, recpool, psA, psO,
              GPOLY, POW16, RSQSEED, RSQNR, SQ, ZPOLY, AF, ts, f32, f32r, bf16):
        from contextlib import ExitStack
        from concourse import mybir
        f16 = mybir.dt.float16
        AxisX = mybir.AxisListType.X
        AluAdd = mybir.AluOpType.add
        ep = ctx.enter_context
        rep = ExitStack()
        rrpool = rep.enter_context(tc.tile_pool(name=f"rr{_rep}", bufs=1))
        vpool = rep.enter_context(tc.tile_pool(name=f"vaug{_rep}", bufs=1))
        qtpool = rep.enter_context(tc.tile_pool(name=f"qt{_rep}", bufs=1))
        onpool = rep.enter_context(tc.tile_pool(name=f"onorm{_rep}", bufs=1))

        ph1 = ExitStack()
        xpool = ph1.enter_context(tc.tile_pool(name=f"xt{_rep}", bufs=3))
        qkpool = ph1.enter_context(tc.tile_pool(name=f"qksb{_rep}", bufs=2))
        scpool = ph1.enter_context(tc.tile_pool(name=f"scr{_rep}", bufs=2))
        sspool = ph1.enter_context(tc.tile_pool(name=f"ssq{_rep}", bufs=1))
        rtpool = ph1.enter_context(tc.tile_pool(name=f"rtmp{_rep}", bufs=4))
        w1pool = ph1.enter_context(tc.tile_pool(name=f"w1{_rep}", bufs=1))
        wqk_t = w1pool.tile([128, NK, 512], f32r, tag="wqk", name="wqk")
        wv_t = w1pool.tile([128, NK, 256], f32r, tag="wv", name="wv")
        nc.sync.dma_start(out=wqk_t[:, :, :], in_=wqkd[:, :])
        nc.sync.dma_start(out=wv_t[:, :, :], in_=wvd[:, :])

        qt_q = [qtpool.tile([128, T], f32r, tag=f"qtq{p}", name=f"qtq{p}") for p in range(2)]
        qt_k = [qtpool.tile([128, T], f32r, tag=f"qtk{p}", name=f"qtk{p}") for p in range(2)]
        v_aug = [vpool.tile([128, 260], bf16, tag=f"v{i}", name=f"v{i}") for i in range(NT)]
        rr = [rrpool.tile([128, 8], f32, tag=f"rr{i}", name=f"rr{i}") for i in range(NT)]
        o_norm = [onpool.tile([128, 256], bf16, tag=f"on{i}", name=f"on{i}") for i in range(NT)]

        # =============== phase 1: projections, norms, transposes ===========
        def emit_proj(i):
            pp = psA.tile([128, 1024], f32, tag="ps", name="ps")
            xt = xpool.tile([128, NK * 128], f32r, tag="xt", name="xt")
            nc.sync.dma_start(out=xt[:, :], in_=xd[i, :, :])
            for kt in range(NK):
                nc.tensor.matmul(pp[:, 0:512], xt[:, ts(kt, 128)], wqk_t[:, kt, :],
                                 start=(kt == 0), stop=(kt == NK - 1))
                nc.tensor.matmul(pp[:, 512:768], xt[:, ts(kt, 128)], wv_t[:, kt, :],
                                 start=(kt == 0), stop=(kt == NK - 1))
            # per-head sum of squares (Q heads then K heads): one wide Act
            # Square to scratch, then a paged DVE X-reduction -> [128, 8]
            ssq = sspool.tile([128, 8], f32, tag=f"ssq{i}", name=f"ssq{i}")
            scr = scpool.tile([128, 8, 64], f32, tag="scr", name="scr")
            nc.scalar.activation(scr[:, :, :], pp[:, 0:512], AF.Square)
            nc.vector.tensor_reduce(ssq[:, :], scr[:, :, :], axis=AxisX,
                                    op=AluAdd)
            # rsqrt of the 8 sums (needed before the Q evacuation below)
            y0 = rtpool.tile([128, 8], f32, tag="rt", name="rt")
            nc.vector._custom_dve(RSQSEED, out=y0[:, :], in0=ssq[:, :],
                                  in1=rc3t[:, :], s0=float(_RC[0]),
                                  s1=float(_RC[1]), imm2=float(_RC[2]))
            y1 = rtpool.tile([128, 8], f32, tag="rt", name="rt")
            nc.vector._custom_dve(RSQNR, out=y1[:, :], in0=y0[:, :],
                                  in1=ssq[:, :], s0=1.5, s1=0.5)
            nc.vector._custom_dve(RSQNR, out=rr[i][:, :], in0=y1[:, :],
                                  in1=ssq[:, :], s0=1.5, s1=0.5)
            # evacuate Q (rq applied via the per-partition scale slot:
            # natural layout has partition == token), K raw, and V (bf16,
            # with ones column)
            qk = qkpool.tile([128, 512], f32r, tag="qk", name="qk")
            for hh in range(4):
                nc.scalar.activation(qk[:, hh * 64:hh * 64 + 64],
                                     pp[:, hh * 64:hh * 64 + 64], AF.Copy,
                                     scale=rr[i][:, hh:hh + 1])
            nc.scalar.activation(qk[:, 256:512], pp[:, 256:512], AF.Copy)
            va = v_aug[i]
            with tc.high_priority():
                nc.vector.memset(va[:], 1.0)
                for hh in range(4):
                    nc.vector.tensor_copy(va[:, hh * 65:hh * 65 + 64],
                                          pp[:, 512 + hh * 64:512 + hh * 64 + 64])
            return qk

        def emit_tpose(i, qk):
            # One full-width (M=128) transpose per head pair: both heads'
            # 64-dim blocks land in psum partitions 0:63 / 64:127 directly.
            # Uses the psO pool (idle during phase 1) so psA's 3 bufs all
            # serve the projection matmuls and keep the PE fed.
            pq = psO.tile([128, 512], f32r, tag="po", name="po")
            for p in range(2):
                nc.tensor.transpose(pq[:, 128 * p:128 * p + 128],
                                    qk[:, 128 * p:128 * p + 128], eye_t[:, :])
                nc.tensor.transpose(pq[:, 256 + 128 * p:384 + 128 * p],
                                    qk[:, 256 + 128 * p:256 + 128 * p + 128],
                                    eye_t[:, :])
            for p in range(2):
                nc.vector.tensor_copy(qt_q[p][:, ts(i, 128)],
                                      pq[:, 128 * p:128 * p + 128])
                nc.vector.tensor_copy(qt_k[p][:, ts(i, 128)],
                                      pq[:, 256 + 128 * p:384 + 128 * p])

        prev = None
        for i in range(NT):
            qk = emit_proj(i)
            if prev is not None:
                emit_tpose(i - 1, prev)
            prev = qk
        emit_tpose(NT - 1, prev)
        ph1.close()

        if _rep == 0:
            nc.sync.dma_start(out=wo_t[:, :, :], in_=wod[:, :])
        wpool = rep.enter_context(tc.tile_pool(name=f"wstrip{_rep}", bufs=NT + 11))
        stpool = rep.enter_context(tc.tile_pool(name=f"st{_rep}", bufs=3))
        ND = 0  # D-path (Ln/Exp) strips per head (0 = all DVE path); NT-ND take the DVE path
        ph2aq = ExitStack()
        apool = ph2aq.enter_context(tc.tile_pool(name=f"atan{_rep}", bufs=3))
        qpool = ph2aq.enter_context(tc.tile_pool(name=f"qpoly{_rep}", bufs=1))
        zpool = (ph2aq.enter_context(tc.tile_pool(name=f"zp{_rep}", bufs=ND))
                 if ND else None)

        # =============== phase 2: per-head attention =======================
        # Head h's scores/chain are interleaved with head h-1's A@V so the
        # PE's A@V burst overlaps the DVE chain of the next head. The strip
        # pool has NT+5 slots; h's first 5 strips are emitted before h-1's
        # A@V (using the spare slots), the rest after, so no instruction
        # ever waits on a slot freed only by a later instruction.
        all_strips = {}
        deferred_d = []

        def emit_scores(h, j):
            # Last ND strips of each head take the Act-heavy D path:
            # one fused DVE z-op, then Ln/Exp on ScalarE (batched per head
            # so the activation table switches only twice per head). The
            # rest take the DVE-heavy A path (GPOLY + POW16). This balances
            # ScalarE vs DVE over the T^2 elementwise work.
            d_path = j >= NT - ND
            p, hp = h // 2, h % 2
            w_strip = wpool.tile([128, T], bf16, tag="w", name="w")
            all_strips[(h, j)] = w_strip
            a_sb = apool.tile([128, T], f16, tag="a", name="a")
            for half in range(NHALF):
                pc = psA.tile([128, 1024], f32, tag="ps", name="ps")
                for sub0 in range(0, TW, 512):
                    sw = min(512, TW - sub0)
                    off = half * TW + sub0
                    nc.tensor.matmul(
                        pc[:, sub0:sub0 + sw],
                        qt_k[p][64 * hp:64 * hp + 64, ts(j, 128)],
                        qt_q[p][64 * hp:64 * hp + 64, off:off + sw],
                        start=True, stop=True)
                nc.scalar.activation(a_sb[:, half * TW:(half + 1) * TW],
                                     pc[:, 0:TW], AF.Arctan,
                                     scale=rr[j][:, 4 + h:5 + h])
            if d_path:
                z_sb = zpool.tile([128, T], f16, tag="z", name="z")
                nc.vector._custom_dve(ZPOLY, out=z_sb[:, :], in0=a_sb[:, :],
                                      s0=float(_ZC[0]), s1=float(_ZC[1]),
                                      imm2=float(_ZC[2]))
                deferred_d.append((z_sb, w_strip))
                return
            q_sb = qpool.tile([128, T], f16, tag="q", name="q")
            nc.vector._custom_dve(GPOLY, out=q_sb[:, :], in0=a_sb[:, :],
                                  in1=k4t[:, :], s0=float(_HC[0]),
                                  s1=float(_HC[1]), imm2=float(_HC[2]))
            nc.vector._custom_dve(POW16, out=w_strip[:, :],
                                  in0=a_sb[:, :], in1=q_sb[:, :], s0=_PI_2)

        def flush_d():
            # w = exp(16*ln(pi/2 + z)). All Lns, then all Exps, so the act
            # table switches at most 3x per head (atan-set -> ln -> exp).
            # Ln runs in place on the z tile (elementwise, same AP). The
            # bias tile is copied on the DVE *after* the head's last z-op,
            # so no Ln becomes schedulable before every arctan of the head
            # has run -- otherwise the list scheduler interleaves Lns into
            # Act idle gaps and pays a 1.3us table load around each one.
            pi2h = recpool.tile([128, 1], f32, tag="pi2h", name="pi2h")
            nc.vector.tensor_copy(pi2h[:, :], pi2t[:, :])
            for z_sb, _ in deferred_d:
                nc.scalar.activation(z_sb[:, :], z_sb[:, :], AF.Ln,
                                     bias=pi2h[:, 0:1])
            for z_sb, w_strip in deferred_d:
                nc.scalar.activation(w_strip[:, :], z_sb[:, :], AF.Exp,
                                     scale=16.0)
            deferred_d.clear()

        def emit_av(h, tcn):
            po = psO.tile([128, 512], f32, tag="po", name="po")
            for j in range(NT):
                nc.tensor.matmul(po[:, 0:65],
                                 all_strips[(h, j)][:, ts(tcn, 128)],
                                 v_aug[j][:, 65 * h:65 * h + 65],
                                 start=(j == 0), stop=(j == NT - 1))
            # evacuate [o|sum] to SBUF, then divide on the idle GPSIMD
            # engine (normalize_recip) -- keeps the row-normalize off the
            # DVE, which is the phase-2 bottleneck engine.
            st = stpool.tile([128, 65], f32, tag="st", name="st")
            nc.scalar.activation(st[:, :], po[:, 0:65], AF.Copy)
            nc.gpsimd.normalize_recip(o_norm[tcn][:, 64 * h:64 * h + 64],
                                      st[:, 0:64], st[:, 64:65])

        # =============== phase 3: transpose o, out-projection ==============
        oT = None  # assigned after ph2 closes

        def emit_otpose(tcn):
            for dp in range(2):
                pt = psO.tile([128, 1024], bf16, tag="po", name="po")
                nc.tensor.transpose(pt[:, 0:128], o_norm[tcn][:, ts(dp, 128)],
                                    eye_b[:, :])
                nc.scalar.activation(oT[dp][:, ts(tcn, 128)], pt[:, 0:128], AF.Copy)

        def emit_oproj(tcn):
            pp = psA.tile([128, 1024], f32, tag="ps", name="ps")
            for dt_ in range(2):
                for pcn in range(2):
                    nc.tensor.matmul(pp[:, ts(pcn, 512)],
                                     oT[dt_][:, ts(tcn, 128)],
                                     wo_t[:, dt_, ts(pcn, 512)],
                                     start=(dt_ == 0), stop=(dt_ == 1))
            osb = outpool.tile([128, 1024], f32, tag="osb", name="osb")
            nc.vector.tensor_copy(osb[:, :], pp[:, :])
            nc.sync.dma_start(out=outd[tcn * 128:(tcn + 1) * 128, :], in_=osb[:, :])

        SPARE = min(11, NT)
        for h in range(4):
            for j in range(SPARE):
                emit_scores(h, j)
            if h > 0:
                for tcn in range(NT):
                    emit_av(h - 1, tcn)
                for j in range(NT):
                    del all_strips[(h - 1, j)]
            for j in range(SPARE, NT):
                emit_scores(h, j)
            flush_d()
        ph2aq.close()
        otpool = rep.enter_context(tc.tile_pool(name=f"ot{_rep}", bufs=1))
        outpool = rep.enter_context(tc.tile_pool(name=f"outsb{_rep}", bufs=2))
        oT = [otpool.tile([128, T], bf16, tag=f"ot{d}", name=f"ot{d}") for d in range(2)]
        for tcn in range(NT):
            emit_av(3, tcn)
            emit_otpose(tcn)
            if tcn >= 1:
                emit_oproj(tcn - 1)
        emit_oproj(NT - 1)
        rep.close()


def _get_nc(T=2048):
    if T not in _BUILT:
        _BUILT[T] = build_nc(T)
    return _BUILT[T]


def _host_inputs(x, Wq, Wk, Wv, Wo, T=2048):
    import ml_dtypes
    f32 = np.float32
    bf16 = ml_dtypes.bfloat16
    in_maps = []
    eye = np.eye(128, dtype=f32)
    per_g = []
    for g in range(4):
        sl = slice(g * 256, (g + 1) * 256)
        wqk = np.ascontiguousarray(
            np.concatenate([Wq[sl].T, Wk[sl].T], axis=1), dtype=f32)  # [1024,512]
        wv = np.ascontiguousarray(Wv[sl].T, dtype=f32)                # [1024,256]
        wo = np.ascontiguousarray(Wo[:, sl].T, dtype=f32)             # [256,1024]
        # pack rows so each SBUF weight tile loads as one contiguous DMA:
        # dest [p, kt, c] <- src row kt*128+p
        wqk = np.ascontiguousarray(
            wqk.reshape(8, 128, 512).transpose(1, 0, 2).reshape(128, 8 * 512))
        wv = np.ascontiguousarray(
            wv.reshape(8, 128, 256).transpose(1, 0, 2).reshape(128, 8 * 256))
        wo = np.ascontiguousarray(
            wo.reshape(2, 128, 1024).transpose(1, 0, 2).reshape(128, 2 * 1024)
        ).astype(bf16)
        per_g.append((wqk, wv, wo))
    for c in range(8):
        b, g = c // 4, c % 4
        xb = np.ascontiguousarray(x[b, :T, :].T, dtype=f32)           # [1024,T]
        xblk = np.ascontiguousarray(
            xb.reshape(8, 128, T // 128, 128).transpose(2, 1, 0, 3).reshape(
                T // 128, 128, 1024))
        wqk, wv, wo = per_g[g]
        in_maps.append({"xb": xblk, "wqk": wqk, "wv": wv, "wo": wo, "eye": eye})
    return in_maps


def kernel(x, Wq, Wk, Wv, Wo, bo):
    from concourse.bass_utils import run_bass_kernel_spmd
    T = 2048
    nc = _get_nc(T)
    in_maps = _host_inputs(np.asarray(x, dtype=np.float32),
                           np.asarray(Wq, dtype=np.float32),
                           np.asarray(Wk, dtype=np.float32),
                           np.asarray(Wv, dtype=np.float32),
                           np.asarray(Wo, dtype=np.float32), T=T)
    res = run_bass_kernel_spmd(nc, in_maps, core_ids=list(range(8)))
    global LAST_RESULT
    LAST_RESULT = res
    outs = [res.results[c]["out"] for c in range(8)]
    bo = np.asarray(bo, dtype=np.float32)
    full = np.empty((2, T, 1024), dtype=np.float32)
    for b in range(2):
        acc = outs[4 * b] + outs[4 * b + 1] + outs[4 * b + 2] + outs[4 * b + 3]
        full[b] = acc + bo
    return full

